# revision 29
# baseline (speedup 1.0000x reference)
"""CIN (Compressed Interaction Network) kernel for Trainium2, 8 NeuronCores.

Reference computation (per layer k, fused einsum):
    xk_new[b,k,d] = sum_{i,j} W[k, i*n+j] * xk[b,i,d] * x0[b,j,d]
    pooled_k[b,:] = sum_d xk_new[b,:,d]
    out = concat(pooled_1, pooled_2, pooled_3)    # (B, 384)

Mapping:
  - Data-parallel over batch: 8 cores x 128 batches each.
  - On-chip layout: partitions = feature index i (H_prev), free dim =
    columns c = (b_local, d) pairs, processed in chunks of C columns.
  - Per layer, loop j in 0..31:
        Y_j = xk (.) broadcast(x0[:, j, :])    (VectorE tensor_tensor, bf16)
        psum[k, c] += W_j^T @ Y_j              (TensorE, K=H_prev contraction)
    Layer 1 packs 4 j's into one K=128 matmul (H_prev=32) by stacking
    4 partition strips: rhs strip s holds x0[i] * x0[4q+s] products.
  - Pooled sums via VectorE reduce over d-groups; intermediate layers
    round-trip PSUM->SBUF in bf16 via ScalarE copies.
  - Output (k, b) tiles are PE-transposed to (b, k) and DMA'd out.
"""

import os
import sys
from contextlib import ExitStack

sys.path.insert(0, "/opt/trn_rl_repo")
os.environ.setdefault("MYCRO_LOCAL_CACHE", "1")

import numpy as np
import ml_dtypes

import concourse.bass as bass
import concourse.tile as tile
from concourse import bacc, mybir
from concourse.bass_utils import run_bass_kernel_spmd
from concourse.masks import make_identity

B, N, D = 1024, 32, 32
H = 128                     # every layer's output features
NCORES = 8
BC = B // NCORES            # 128 batches per core
COLS = BC * D               # 4096 columns per core
C = 1024                    # chunk columns (32 batches x 32 d)
NB = C // D                 # batches per chunk
NCHUNK = COLS // C
MMN = 512                   # matmul moving free dim (one PSUM bank of fp32)
BF = mybir.dt.bfloat16
F32 = mybir.dt.float32

_CACHE = {}


def _dap(handle, offset, dims):
    a = handle[:]
    return bass.AP(tensor=a.tensor, offset=offset, ap=dims)


def _build_program():
    nc = bacc.Bacc(
        "TRN2", target_bir_lowering=False, debug=False, num_devices=NCORES
    )
    xr = nc.declare_dram_parameter("xr", [128, COLS], BF, isOutput=False)
    f4a = nc.declare_dram_parameter("f4a", [8, 128, COLS], BF, isOutput=False)
    fja = nc.declare_dram_parameter("fja", [N, 128, COLS], BF, isOutput=False)
    w0p = nc.declare_dram_parameter("w0p", [8, 128, H], BF, isOutput=False)
    w1p = nc.declare_dram_parameter("w1p", [N, H, H], BF, isOutput=False)
    w2p = nc.declare_dram_parameter("w2p", [N, H, H], BF, isOutput=False)
    out = nc.declare_dram_parameter("out", [BC, 3 * H], F32, isOutput=True)

    with tile.TileContext(nc) as tc, ExitStack() as ctx:
        singles = ctx.enter_context(tc.tile_pool(name="singles", bufs=1))
        f4pool = ctx.enter_context(tc.tile_pool(name="f4pool", bufs=1))
        fpool = ctx.enter_context(tc.tile_pool(name="fpool", bufs=1))
        x0pool = ctx.enter_context(tc.tile_pool(name="x0pool", bufs=1))
        xpool = ctx.enter_context(tc.tile_pool(name="xpool", bufs=3))
        ypool = ctx.enter_context(tc.tile_pool(name="ypool", bufs=5))
        pspool = ctx.enter_context(tc.tile_pool(name="ps", bufs=4, space="PSUM"))

        # --- weights, identity, persistent accumulators ---
        w0t = singles.tile([128, 8, H], BF)
        nc.sync.dma_start(out=w0t[:], in_=_dap(w0p, 0, [[H, 128], [128 * H, 8], [1, H]]))
        w1t = singles.tile([128, N, H], BF)
        w2t = singles.tile([128, N, H], BF)
        ident = singles.tile([128, 128], F32)
        make_identity(nc, ident[:])
        pooled = singles.tile([128, 3, BC], F32)
        out_sb = singles.tile([128, 3 * H], F32)

        def bcast4(tile_ap):
            # (128, C) tile read as (128, 4, C) with the j-dim broadcast
            return bass.AP(
                tensor=tile_ap.tensor,
                offset=tile_ap.offset,
                ap=[tile_ap.ap[0], [0, 4], tile_ap.ap[1]],
            )

        NH = N // 2  # j's per fjt half-tile

        def load_factors(ich):
            x0r = x0pool.tile([128, C], BF, tag="x0r")
            nc.scalar.dma_start(
                out=x0r[:], in_=_dap(xr, ich * C, [[COLS, 128], [1, C]])
            )
            f4t = f4pool.tile([128, 8, C], BF, tag="f4")
            nc.scalar.dma_start(
                out=f4t[:],
                in_=_dap(f4a, ich * C, [[COLS, 128], [128 * COLS, 8], [1, C]]),
            )
            return x0r, f4t

        def load_fj(ich):
            fjA = fpool.tile([128, NH, C], BF, tag="fjA")
            nc.sync.dma_start(
                out=fjA[:],
                in_=_dap(fja, ich * C, [[COLS, 128], [128 * COLS, NH], [1, C]]),
            )
            fjB = fpool.tile([128, NH, C], BF, tag="fjB")
            nc.scalar.dma_start(
                out=fjB[:],
                in_=_dap(
                    fja,
                    NH * 128 * COLS + ich * C,
                    [[COLS, 128], [128 * COLS, NH], [1, C]],
                ),
            )
            return fjA, fjB

        def layer1(x0r, f4t):
            ps1 = pspool.tile([128, C], F32, tag="ps")
            for g in range(2):
                y = ypool.tile([128, 4, C], BF, tag="y")
                nc.vector.tensor_mul(
                    y[:], bcast4(x0r[:]), f4t[:, 4 * g : 4 * (g + 1), :]
                )
                for ql in range(4):
                    q = 4 * g + ql
                    for t in range(C // MMN):
                        nc.tensor.matmul(
                            ps1[:, MMN * t : MMN * (t + 1)],
                            lhsT=w0t[:, q, :],
                            rhs=y[:, ql, MMN * t : MMN * (t + 1)],
                            start=(q == 0),
                            stop=(q == 7),
                        )
            x1 = xpool.tile([128, C], BF, tag="x")
            nc.scalar.copy(out=x1[:], in_=ps1[:])
            return ps1, x1

        def reduce_ps(ps, layer, ich):
            nc.vector.reduce_sum(
                out=pooled[:, layer, ich * NB : (ich + 1) * NB],
                in_=ps[:].rearrange("p (b d) -> p b d", d=D),
                axis=mybir.AxisListType.X,
            )

        def quad(xk, wt, ps, fjA, fjB, g):
            j0 = 4 * g
            fh, fo = (fjA, j0) if j0 < NH else (fjB, j0 - NH)
            y = ypool.tile([128, 4, C], BF, tag="y")
            nc.vector.tensor_mul(y[:], bcast4(xk[:]), fh[:, fo : fo + 4, :])
            for jl in range(4):
                j = j0 + jl
                for t in range(C // MMN):
                    nc.tensor.matmul(
                        ps[:, MMN * t : MMN * (t + 1)],
                        lhsT=wt[:, j, :],
                        rhs=y[:, jl, MMN * t : MMN * (t + 1)],
                        start=(j == 0),
                        stop=(j == N - 1),
                    )

        # Sequential PSUM groups, boundary-overlapped: chunk k+1's L1 is
        # emitted between chunk k's L2 and L3 (absorbing the L2 PE tail),
        # and every pooled reduce is emitted ~2 TTs after its group's stop
        # so the in-order DVE never waits on a PE accumulation tail.
        def quad(xk, wt, ps, fjA, fjB, g, rds):
            j0 = 4 * g
            fh, fo = (fjA, j0) if j0 < NH else (fjB, j0 - NH)
            y = ypool.tile([128, 4, C], BF, tag="y")
            nc.vector.tensor_mul(y[:], bcast4(xk[:]), fh[:, fo : fo + 4, :])
            if g == 2:
                for ps_, layer_, ich_ in rds:
                    reduce_ps(ps_, layer_, ich_)
                rds.clear()
            for jl in range(4):
                j = j0 + jl
                for t in range(C // MMN):
                    nc.tensor.matmul(
                        ps[:, MMN * t : MMN * (t + 1)],
                        lhsT=wt[:, j, :],
                        rhs=y[:, jl, MMN * t : MMN * (t + 1)],
                        start=(j == 0),
                        stop=(j == N - 1),
                    )

        x0rn, f4tn = load_factors(0)
        fj = {0: load_fj(0)}
        nc.scalar.dma_start(out=w1t[:], in_=_dap(w1p, 0, [[H, 128], [128 * H, N], [1, H]]))
        nc.sync.dma_start(out=w2t[:], in_=_dap(w2p, 0, [[H, 128], [128 * H, N], [1, H]]))
        ps1_next, x1_next = layer1(x0rn, f4tn)
        rds = [(ps1_next, 0, 0)]
        x1 = {0: x1_next}

        for k in range(NCHUNK):
            # ---- layer 2 of chunk k ----
            ps2 = pspool.tile([128, C], F32, tag="ps", name=f"ps2_{k}")
            for g in range(8):
                quad(x1[k], w1t, ps2, fj[k][0], fj[k][1], g, rds)
                if g == 0 and k + 1 < NCHUNK:
                    x0rn, f4tn = load_factors(k + 1)
                    fj[k + 1] = load_fj(k + 1)
            x2 = xpool.tile([128, C], BF, tag="x", name=f"x2_{k}")
            nc.scalar.copy(out=x2[:], in_=ps2[:])
            # ---- layer 1 of chunk k+1 (independent filler work) ----
            if k + 1 < NCHUNK:
                ps1_next, x1[k + 1] = layer1(x0rn, f4tn)
                rds.append((ps1_next, 0, k + 1))
            rds.append((ps2, 1, k))
            # ---- layer 3 of chunk k ----
            ps3 = pspool.tile([128, C], F32, tag="ps", name=f"ps3_{k}")
            for g in range(8):
                quad(x2, w2t, ps3, fj[k][0], fj[k][1], g, rds)
            rds.append((ps3, 2, k))
        for ps_, layer_, ich_ in rds:
            reduce_ps(ps_, layer_, ich_)

        # ---- finalize: transpose pooled (k, b) -> (b, k), store ----
        for layer in range(3):
            tp = pspool.tile([128, 128], F32, tag="ps", name=f"tp_{layer}")
            nc.tensor.transpose(tp[:], pooled[:, layer, :], ident[:])
            nc.scalar.copy(out=out_sb[:, H * layer : H * (layer + 1)], in_=tp[:])
        nc.sync.dma_start(out=out[:], in_=out_sb[:])

    nc.compile()
    return nc


def _prep_inputs(x0, w0, w1, w2):
    bf = ml_dtypes.bfloat16
    x0b = np.ascontiguousarray(x0.astype(bf))
    # w0: (N*N, H) -> (i, j, k) -> quad-packed (8, 4*32, H), p = jl*32 + i
    w0r = w0.reshape(N, N, H).transpose(1, 0, 2)          # (j, i, k)
    w0q = np.ascontiguousarray(
        w0r.reshape(8, 4, N, H).reshape(8, 128, H).astype(bf)
    )
    w1r = np.ascontiguousarray(
        w1.reshape(H, N, H).transpose(1, 0, 2).astype(bf)  # (j, i, k)
    )
    w2r = np.ascontiguousarray(
        w2.reshape(H, N, H).transpose(1, 0, 2).astype(bf)
    )
    return x0b, w0q, w1r, w2r


def _get_compiled():
    if "nc" not in _CACHE:
        _CACHE["nc"] = _build_program()
    return _CACHE["nc"]


def run(x0, w0, w1, w2, trace=False):
    nc = _get_compiled()
    x0b, w0q, w1r, w2r = _prep_inputs(
        np.asarray(x0, np.float32),
        np.asarray(w0, np.float32),
        np.asarray(w1, np.float32),
        np.asarray(w2, np.float32),
    )
    core_ids = list(range(NCORES))
    quad_rows = np.repeat(np.arange(N), 32).reshape(8, 128)
    in_maps = []
    for c in core_ids:
        shard = np.ascontiguousarray(x0b[c * BC : (c + 1) * BC])
        x0t = np.ascontiguousarray(shard.transpose(1, 0, 2).reshape(N, COLS))
        in_maps.append(
            {
                "xr": np.ascontiguousarray(np.tile(x0t, (4, 1))),
                "f4a": np.ascontiguousarray(x0t[quad_rows]),
                "fja": np.ascontiguousarray(
                    np.broadcast_to(x0t[:, None, :], (N, 128, COLS))
                ),
                "w0p": w0q,
                "w1p": w1r,
                "w2p": w2r,
            }
        )
    res = run_bass_kernel_spmd(nc, in_maps, core_ids, trace=trace)
    outs = [np.asarray(res.results[c]["out"], np.float32) for c in core_ids]
    return np.concatenate(outs, axis=0), res


def kernel(x0, w0, w1, w2):
    full, _ = run(x0, w0, w1, w2, trace=False)
    return full


# revision 30
# speedup vs baseline: 1.0667x; 1.0667x over previous
"""CIN (Compressed Interaction Network) kernel for Trainium2, 8 NeuronCores.

Reference computation (per layer k, fused einsum):
    xk_new[b,k,d] = sum_{i,j} W[k, i*n+j] * xk[b,i,d] * x0[b,j,d]
    pooled_k[b,:] = sum_d xk_new[b,:,d]
    out = concat(pooled_1, pooled_2, pooled_3)    # (B, 384)

Mapping:
  - Data-parallel over batch: 8 cores x 128 batches each.
  - On-chip layout: partitions = feature index i (H_prev), free dim =
    columns c = (b_local, d) pairs, processed in chunks of C columns.
  - Per layer, loop j in 0..31:
        Y_j = xk (.) broadcast(x0[:, j, :])    (VectorE tensor_tensor, bf16)
        psum[k, c] += W_j^T @ Y_j              (TensorE, K=H_prev contraction)
    Layer 1 packs 4 j's into one K=128 matmul (H_prev=32) by stacking
    4 partition strips: rhs strip s holds x0[i] * x0[4q+s] products.
  - Pooled sums via VectorE reduce over d-groups; intermediate layers
    round-trip PSUM->SBUF in bf16 via ScalarE copies.
  - Output (k, b) tiles are PE-transposed to (b, k) and DMA'd out.
"""

import os
import sys
from contextlib import ExitStack

sys.path.insert(0, "/opt/trn_rl_repo")
os.environ.setdefault("MYCRO_LOCAL_CACHE", "1")

import numpy as np
import ml_dtypes

import concourse.bass as bass
import concourse.tile as tile
from concourse import bacc, mybir
from concourse.bass_utils import run_bass_kernel_spmd
from concourse.masks import make_identity

B, N, D = 1024, 32, 32
H = 128                     # every layer's output features
NCORES = 8
BC = B // NCORES            # 128 batches per core
COLS = BC * D               # 4096 columns per core
C = 1024                    # chunk columns (32 batches x 32 d)
NB = C // D                 # batches per chunk
NCHUNK = COLS // C
MMN = 512                   # matmul moving free dim (one PSUM bank of fp32)
BF = mybir.dt.bfloat16
F32 = mybir.dt.float32

_CACHE = {}


def _dap(handle, offset, dims):
    a = handle[:]
    return bass.AP(tensor=a.tensor, offset=offset, ap=dims)


def _build_program():
    nc = bacc.Bacc(
        "TRN2", target_bir_lowering=False, debug=False, num_devices=NCORES
    )
    xr = nc.declare_dram_parameter("xr", [128, COLS], BF, isOutput=False)
    f4a = nc.declare_dram_parameter("f4a", [8, 128, COLS], BF, isOutput=False)
    fja = nc.declare_dram_parameter("fja", [N, 128, COLS], BF, isOutput=False)
    w0p = nc.declare_dram_parameter("w0p", [8, 128, H], BF, isOutput=False)
    w1p = nc.declare_dram_parameter("w1p", [N, H, H], BF, isOutput=False)
    w2p = nc.declare_dram_parameter("w2p", [N, H, H], BF, isOutput=False)
    out = nc.declare_dram_parameter("out", [BC, 3 * H], F32, isOutput=True)

    with tile.TileContext(nc) as tc, ExitStack() as ctx:
        singles = ctx.enter_context(tc.tile_pool(name="singles", bufs=1))
        f4pool = ctx.enter_context(tc.tile_pool(name="f4pool", bufs=1))
        fpool = ctx.enter_context(tc.tile_pool(name="fpool", bufs=1))
        x0pool = ctx.enter_context(tc.tile_pool(name="x0pool", bufs=1))
        xpool = ctx.enter_context(tc.tile_pool(name="xpool", bufs=3))
        ypool = ctx.enter_context(tc.tile_pool(name="ypool", bufs=5))
        pspool = ctx.enter_context(tc.tile_pool(name="ps", bufs=4, space="PSUM"))

        # --- weights, identity, persistent accumulators ---
        w0t = singles.tile([128, 8, H], BF)
        nc.sync.dma_start(out=w0t[:], in_=_dap(w0p, 0, [[H, 128], [128 * H, 8], [1, H]]))
        w1t = singles.tile([128, N, H], BF)
        w2t = singles.tile([128, N, H], BF)
        ident = singles.tile([128, 128], F32)
        make_identity(nc, ident[:])
        pooled = singles.tile([128, 3, BC], F32)
        out_sb = singles.tile([128, 3 * H], F32)

        def bcast4(tile_ap):
            # (128, C) tile read as (128, 4, C) with the j-dim broadcast
            return bass.AP(
                tensor=tile_ap.tensor,
                offset=tile_ap.offset,
                ap=[tile_ap.ap[0], [0, 4], tile_ap.ap[1]],
            )

        NH = N // 2  # j's per fjt half-tile

        def load_factors(ich):
            x0r = x0pool.tile([128, C], BF, tag="x0r")
            nc.scalar.dma_start(
                out=x0r[:], in_=_dap(xr, ich * C, [[COLS, 128], [1, C]])
            )
            f4t = f4pool.tile([128, 8, C], BF, tag="f4")
            nc.scalar.dma_start(
                out=f4t[:],
                in_=_dap(f4a, ich * C, [[COLS, 128], [128 * COLS, 8], [1, C]]),
            )
            return x0r, f4t

        NQ = N // 4  # j's per quarter tile

        def load_fj(ich):
            # four quarter tiles, alternating rings, so the first j's of a
            # chunk land ~4x sooner than a monolithic load
            tiles = []
            for qt in range(4):
                t = fpool.tile([128, NQ, C], BF, tag=f"fjq{qt}", name=f"fj{ich}_{qt}")
                eng = nc.sync if qt % 2 == 0 else nc.scalar
                eng.dma_start(
                    out=t[:],
                    in_=_dap(
                        fja,
                        qt * NQ * 128 * COLS + ich * C,
                        [[COLS, 128], [128 * COLS, NQ], [1, C]],
                    ),
                )
                tiles.append(t)
            return tiles

        def layer1(x0r, f4t):
            ps1 = pspool.tile([128, C], F32, tag="ps")
            for g in range(2):
                y = ypool.tile([128, 4, C], BF, tag="y")
                nc.vector.tensor_mul(
                    y[:], bcast4(x0r[:]), f4t[:, 4 * g : 4 * (g + 1), :]
                )
                for ql in range(4):
                    q = 4 * g + ql
                    for t in range(C // MMN):
                        nc.tensor.matmul(
                            ps1[:, MMN * t : MMN * (t + 1)],
                            lhsT=w0t[:, q, :],
                            rhs=y[:, ql, MMN * t : MMN * (t + 1)],
                            start=(q == 0),
                            stop=(q == 7),
                        )
            x1 = xpool.tile([128, C], BF, tag="x")
            nc.scalar.copy(out=x1[:], in_=ps1[:])
            return ps1, x1

        def reduce_ps(ps, layer, ich):
            nc.vector.reduce_sum(
                out=pooled[:, layer, ich * NB : (ich + 1) * NB],
                in_=ps[:].rearrange("p (b d) -> p b d", d=D),
                axis=mybir.AxisListType.X,
            )

        def quad(xk, wt, ps, fjA, fjB, g):
            j0 = 4 * g
            fh, fo = (fjA, j0) if j0 < NH else (fjB, j0 - NH)
            y = ypool.tile([128, 4, C], BF, tag="y")
            nc.vector.tensor_mul(y[:], bcast4(xk[:]), fh[:, fo : fo + 4, :])
            for jl in range(4):
                j = j0 + jl
                for t in range(C // MMN):
                    nc.tensor.matmul(
                        ps[:, MMN * t : MMN * (t + 1)],
                        lhsT=wt[:, j, :],
                        rhs=y[:, jl, MMN * t : MMN * (t + 1)],
                        start=(j == 0),
                        stop=(j == N - 1),
                    )

        # Sequential PSUM groups, boundary-overlapped: chunk k+1's L1 is
        # emitted between chunk k's L2 and L3 (absorbing the L2 PE tail),
        # and every pooled reduce is emitted ~2 TTs after its group's stop
        # so the in-order DVE never waits on a PE accumulation tail.
        def quad(xk, wt, ps, fjt, g, rds):
            j0 = 4 * g
            fh, fo = fjt[j0 // NQ], j0 % NQ
            y = ypool.tile([128, 4, C], BF, tag="y")
            nc.vector.tensor_mul(y[:], bcast4(xk[:]), fh[:, fo : fo + 4, :])
            if g == 2:
                for ps_, layer_, ich_ in rds:
                    reduce_ps(ps_, layer_, ich_)
                rds.clear()
            for jl in range(4):
                j = j0 + jl
                for t in range(C // MMN):
                    nc.tensor.matmul(
                        ps[:, MMN * t : MMN * (t + 1)],
                        lhsT=wt[:, j, :],
                        rhs=y[:, jl, MMN * t : MMN * (t + 1)],
                        start=(j == 0),
                        stop=(j == N - 1),
                    )

        x0rn, f4tn = load_factors(0)
        fj = {0: load_fj(0)}
        nc.scalar.dma_start(out=w1t[:], in_=_dap(w1p, 0, [[H, 128], [128 * H, N], [1, H]]))
        nc.sync.dma_start(out=w2t[:], in_=_dap(w2p, 0, [[H, 128], [128 * H, N], [1, H]]))
        ps1_next, x1_next = layer1(x0rn, f4tn)
        rds = [(ps1_next, 0, 0)]
        x1 = {0: x1_next}

        for k in range(NCHUNK):
            # ---- layer 2 of chunk k ----
            ps2 = pspool.tile([128, C], F32, tag="ps", name=f"ps2_{k}")
            for g in range(8):
                quad(x1[k], w1t, ps2, fj[k], g, rds)
                if g == 0 and k + 1 < NCHUNK:
                    x0rn, f4tn = load_factors(k + 1)
                    fj[k + 1] = load_fj(k + 1)
            x2 = xpool.tile([128, C], BF, tag="x", name=f"x2_{k}")
            nc.scalar.copy(out=x2[:], in_=ps2[:])
            # ---- layer 1 of chunk k+1 (independent filler work) ----
            if k + 1 < NCHUNK:
                ps1_next, x1[k + 1] = layer1(x0rn, f4tn)
                rds.append((ps1_next, 0, k + 1))
            rds.append((ps2, 1, k))
            # ---- layer 3 of chunk k ----
            ps3 = pspool.tile([128, C], F32, tag="ps", name=f"ps3_{k}")
            for g in range(8):
                quad(x2, w2t, ps3, fj[k], g, rds)
            rds.append((ps3, 2, k))
        for ps_, layer_, ich_ in rds:
            reduce_ps(ps_, layer_, ich_)

        # ---- finalize: transpose pooled (k, b) -> (b, k), store ----
        for layer in range(3):
            tp = pspool.tile([128, 128], F32, tag="ps", name=f"tp_{layer}")
            nc.tensor.transpose(tp[:], pooled[:, layer, :], ident[:])
            nc.scalar.copy(out=out_sb[:, H * layer : H * (layer + 1)], in_=tp[:])
        nc.sync.dma_start(out=out[:], in_=out_sb[:])

    nc.compile()
    return nc


def _prep_inputs(x0, w0, w1, w2):
    bf = ml_dtypes.bfloat16
    x0b = np.ascontiguousarray(x0.astype(bf))
    # w0: (N*N, H) -> (i, j, k) -> quad-packed (8, 4*32, H), p = jl*32 + i
    w0r = w0.reshape(N, N, H).transpose(1, 0, 2)          # (j, i, k)
    w0q = np.ascontiguousarray(
        w0r.reshape(8, 4, N, H).reshape(8, 128, H).astype(bf)
    )
    w1r = np.ascontiguousarray(
        w1.reshape(H, N, H).transpose(1, 0, 2).astype(bf)  # (j, i, k)
    )
    w2r = np.ascontiguousarray(
        w2.reshape(H, N, H).transpose(1, 0, 2).astype(bf)
    )
    return x0b, w0q, w1r, w2r


def _get_compiled():
    if "nc" not in _CACHE:
        _CACHE["nc"] = _build_program()
    return _CACHE["nc"]


def run(x0, w0, w1, w2, trace=False):
    nc = _get_compiled()
    x0b, w0q, w1r, w2r = _prep_inputs(
        np.asarray(x0, np.float32),
        np.asarray(w0, np.float32),
        np.asarray(w1, np.float32),
        np.asarray(w2, np.float32),
    )
    core_ids = list(range(NCORES))
    quad_rows = np.repeat(np.arange(N), 32).reshape(8, 128)
    in_maps = []
    for c in core_ids:
        shard = np.ascontiguousarray(x0b[c * BC : (c + 1) * BC])
        x0t = np.ascontiguousarray(shard.transpose(1, 0, 2).reshape(N, COLS))
        in_maps.append(
            {
                "xr": np.ascontiguousarray(np.tile(x0t, (4, 1))),
                "f4a": np.ascontiguousarray(x0t[quad_rows]),
                "fja": np.ascontiguousarray(
                    np.broadcast_to(x0t[:, None, :], (N, 128, COLS))
                ),
                "w0p": w0q,
                "w1p": w1r,
                "w2p": w2r,
            }
        )
    res = run_bass_kernel_spmd(nc, in_maps, core_ids, trace=trace)
    outs = [np.asarray(res.results[c]["out"], np.float32) for c in core_ids]
    return np.concatenate(outs, axis=0), res


def kernel(x0, w0, w1, w2):
    full, _ = run(x0, w0, w1, w2, trace=False)
    return full


# revision 31
# speedup vs baseline: 1.1050x; 1.0359x over previous
"""CIN (Compressed Interaction Network) kernel for Trainium2, 8 NeuronCores.

Reference computation (per layer k, fused einsum):
    xk_new[b,k,d] = sum_{i,j} W[k, i*n+j] * xk[b,i,d] * x0[b,j,d]
    pooled_k[b,:] = sum_d xk_new[b,:,d]
    out = concat(pooled_1, pooled_2, pooled_3)    # (B, 384)

Mapping:
  - Data-parallel over batch: 8 cores x 128 batches each.
  - On-chip layout: partitions = feature index i (H_prev), free dim =
    columns c = (b_local, d) pairs, processed in chunks of C columns.
  - Per layer, loop j in 0..31:
        Y_j = xk (.) broadcast(x0[:, j, :])    (VectorE tensor_tensor, bf16)
        psum[k, c] += W_j^T @ Y_j              (TensorE, K=H_prev contraction)
    Layer 1 packs 4 j's into one K=128 matmul (H_prev=32) by stacking
    4 partition strips: rhs strip s holds x0[i] * x0[4q+s] products.
  - Pooled sums via VectorE reduce over d-groups; intermediate layers
    round-trip PSUM->SBUF in bf16 via ScalarE copies.
  - Output (k, b) tiles are PE-transposed to (b, k) and DMA'd out.
"""

import os
import sys
from contextlib import ExitStack

sys.path.insert(0, "/opt/trn_rl_repo")
os.environ.setdefault("MYCRO_LOCAL_CACHE", "1")

import numpy as np
import ml_dtypes

import concourse.bass as bass
import concourse.tile as tile
from concourse import bacc, mybir
from concourse.bass_utils import run_bass_kernel_spmd
from concourse.masks import make_identity

B, N, D = 1024, 32, 32
H = 128                     # every layer's output features
NCORES = 8
BC = B // NCORES            # 128 batches per core
COLS = BC * D               # 4096 columns per core
C = 1024                    # chunk columns (32 batches x 32 d)
NB = C // D                 # batches per chunk
NCHUNK = COLS // C
MMN = 512                   # matmul moving free dim (one PSUM bank of fp32)
BF = mybir.dt.bfloat16
F32 = mybir.dt.float32

_CACHE = {}


def _dap(handle, offset, dims):
    a = handle[:]
    return bass.AP(tensor=a.tensor, offset=offset, ap=dims)


def _build_program():
    nc = bacc.Bacc(
        "TRN2", target_bir_lowering=False, debug=False, num_devices=NCORES
    )
    xr = nc.declare_dram_parameter("xr", [128, COLS], BF, isOutput=False)
    f4a = nc.declare_dram_parameter("f4a", [8, 128, COLS], BF, isOutput=False)
    fja = nc.declare_dram_parameter("fja", [N, 128, COLS], BF, isOutput=False)
    w0p = nc.declare_dram_parameter("w0p", [8, 128, H], BF, isOutput=False)
    w1p = nc.declare_dram_parameter("w1p", [N, H, H], BF, isOutput=False)
    w2p = nc.declare_dram_parameter("w2p", [N, H, H], BF, isOutput=False)
    out = nc.declare_dram_parameter("out", [BC, 3 * H], F32, isOutput=True)

    with tile.TileContext(nc) as tc, ExitStack() as ctx:
        singles = ctx.enter_context(tc.tile_pool(name="singles", bufs=1))
        f4pool = ctx.enter_context(tc.tile_pool(name="f4pool", bufs=1))
        fpool = ctx.enter_context(tc.tile_pool(name="fpool", bufs=1))
        x0pool = ctx.enter_context(tc.tile_pool(name="x0pool", bufs=1))
        xpool = ctx.enter_context(tc.tile_pool(name="xpool", bufs=3))
        ypool = ctx.enter_context(tc.tile_pool(name="ypool", bufs=5))
        pspool = ctx.enter_context(tc.tile_pool(name="ps", bufs=4, space="PSUM"))

        # --- weights, identity, persistent accumulators ---
        w0t = singles.tile([128, 8, H], BF)
        nc.sync.dma_start(out=w0t[:], in_=_dap(w0p, 0, [[H, 128], [128 * H, 8], [1, H]]))
        w1t = singles.tile([128, N, H], BF)
        w2t = singles.tile([128, N, H], BF)
        ident = singles.tile([128, 128], F32)
        make_identity(nc, ident[:])
        pooled = singles.tile([128, 3, BC], F32)
        out_sb = singles.tile([128, 3 * H], F32)

        def bcast4(tile_ap):
            # (128, C) tile read as (128, 4, C) with the j-dim broadcast
            return bass.AP(
                tensor=tile_ap.tensor,
                offset=tile_ap.offset,
                ap=[tile_ap.ap[0], [0, 4], tile_ap.ap[1]],
            )

        NH = N // 2  # j's per fjt half-tile

        def load_factors(ich):
            x0r = x0pool.tile([128, C], BF, tag="x0r")
            nc.scalar.dma_start(
                out=x0r[:], in_=_dap(xr, ich * C, [[COLS, 128], [1, C]])
            )
            f4t = []
            for h in range(2):
                t = f4pool.tile([128, 4, C], BF, tag=f"f4{h}", name=f"f4_{ich}_{h}")
                nc.scalar.dma_start(
                    out=t[:],
                    in_=_dap(
                        f4a,
                        4 * h * 128 * COLS + ich * C,
                        [[COLS, 128], [128 * COLS, 4], [1, C]],
                    ),
                )
                f4t.append(t)
            return x0r, f4t

        NQ = N // 4  # j's per quarter tile

        def load_fj(ich):
            # four quarter tiles, alternating rings, so the first j's of a
            # chunk land ~4x sooner than a monolithic load
            tiles = []
            for qt in range(4):
                t = fpool.tile([128, NQ, C], BF, tag=f"fjq{qt}", name=f"fj{ich}_{qt}")
                eng = nc.sync if qt % 2 == 0 else nc.scalar
                eng.dma_start(
                    out=t[:],
                    in_=_dap(
                        fja,
                        qt * NQ * 128 * COLS + ich * C,
                        [[COLS, 128], [128 * COLS, NQ], [1, C]],
                    ),
                )
                tiles.append(t)
            return tiles

        def layer1(x0r, f4t):
            ps1 = pspool.tile([128, C], F32, tag="ps")
            for g in range(2):
                y = ypool.tile([128, 4, C], BF, tag="y")
                nc.vector.tensor_mul(y[:], bcast4(x0r[:]), f4t[g][:])
                for ql in range(4):
                    q = 4 * g + ql
                    for t in range(C // MMN):
                        nc.tensor.matmul(
                            ps1[:, MMN * t : MMN * (t + 1)],
                            lhsT=w0t[:, q, :],
                            rhs=y[:, ql, MMN * t : MMN * (t + 1)],
                            start=(q == 0),
                            stop=(q == 7),
                        )
            x1 = xpool.tile([128, C], BF, tag="x")
            nc.scalar.copy(out=x1[:], in_=ps1[:])
            return ps1, x1

        def reduce_ps(ps, layer, ich):
            nc.vector.reduce_sum(
                out=pooled[:, layer, ich * NB : (ich + 1) * NB],
                in_=ps[:].rearrange("p (b d) -> p b d", d=D),
                axis=mybir.AxisListType.X,
            )

        def quad(xk, wt, ps, fjA, fjB, g):
            j0 = 4 * g
            fh, fo = (fjA, j0) if j0 < NH else (fjB, j0 - NH)
            y = ypool.tile([128, 4, C], BF, tag="y")
            nc.vector.tensor_mul(y[:], bcast4(xk[:]), fh[:, fo : fo + 4, :])
            for jl in range(4):
                j = j0 + jl
                for t in range(C // MMN):
                    nc.tensor.matmul(
                        ps[:, MMN * t : MMN * (t + 1)],
                        lhsT=wt[:, j, :],
                        rhs=y[:, jl, MMN * t : MMN * (t + 1)],
                        start=(j == 0),
                        stop=(j == N - 1),
                    )

        # Sequential PSUM groups, boundary-overlapped: chunk k+1's L1 is
        # emitted between chunk k's L2 and L3 (absorbing the L2 PE tail),
        # and every pooled reduce is emitted ~2 TTs after its group's stop
        # so the in-order DVE never waits on a PE accumulation tail.
        def quad(xk, wt, ps, fjt, g, rds):
            j0 = 4 * g
            fh, fo = fjt[j0 // NQ], j0 % NQ
            y = ypool.tile([128, 4, C], BF, tag="y")
            nc.vector.tensor_mul(y[:], bcast4(xk[:]), fh[:, fo : fo + 4, :])
            if g == 2:
                for ps_, layer_, ich_ in rds:
                    reduce_ps(ps_, layer_, ich_)
                rds.clear()
            for jl in range(4):
                j = j0 + jl
                for t in range(C // MMN):
                    nc.tensor.matmul(
                        ps[:, MMN * t : MMN * (t + 1)],
                        lhsT=wt[:, j, :],
                        rhs=y[:, jl, MMN * t : MMN * (t + 1)],
                        start=(j == 0),
                        stop=(j == N - 1),
                    )

        x0rn, f4tn = load_factors(0)
        fj = {0: load_fj(0)}
        nc.scalar.dma_start(out=w1t[:], in_=_dap(w1p, 0, [[H, 128], [128 * H, N], [1, H]]))
        nc.sync.dma_start(out=w2t[:], in_=_dap(w2p, 0, [[H, 128], [128 * H, N], [1, H]]))
        ps1_next, x1_next = layer1(x0rn, f4tn)
        rds = [(ps1_next, 0, 0)]
        x1 = {0: x1_next}

        for k in range(NCHUNK):
            # ---- layer 2 of chunk k ----
            ps2 = pspool.tile([128, C], F32, tag="ps", name=f"ps2_{k}")
            for g in range(8):
                quad(x1[k], w1t, ps2, fj[k], g, rds)
                if g == 0 and k + 1 < NCHUNK:
                    x0rn, f4tn = load_factors(k + 1)
                    fj[k + 1] = load_fj(k + 1)
            x2 = xpool.tile([128, C], BF, tag="x", name=f"x2_{k}")
            nc.scalar.copy(out=x2[:], in_=ps2[:])
            # ---- layer 1 of chunk k+1 (independent filler work) ----
            if k + 1 < NCHUNK:
                ps1_next, x1[k + 1] = layer1(x0rn, f4tn)
                rds.append((ps1_next, 0, k + 1))
            rds.append((ps2, 1, k))
            # ---- layer 3 of chunk k ----
            ps3 = pspool.tile([128, C], F32, tag="ps", name=f"ps3_{k}")
            for g in range(8):
                quad(x2, w2t, ps3, fj[k], g, rds)
            rds.append((ps3, 2, k))
        for ps_, layer_, ich_ in rds:
            reduce_ps(ps_, layer_, ich_)

        # ---- finalize: transpose pooled (k, b) -> (b, k), store ----
        for layer in range(3):
            tp = pspool.tile([128, 128], F32, tag="ps", name=f"tp_{layer}")
            nc.tensor.transpose(tp[:], pooled[:, layer, :], ident[:])
            nc.scalar.copy(out=out_sb[:, H * layer : H * (layer + 1)], in_=tp[:])
        nc.sync.dma_start(out=out[:], in_=out_sb[:])

    nc.compile()
    return nc


def _prep_inputs(x0, w0, w1, w2):
    bf = ml_dtypes.bfloat16
    x0b = np.ascontiguousarray(x0.astype(bf))
    # w0: (N*N, H) -> (i, j, k) -> quad-packed (8, 4*32, H), p = jl*32 + i
    w0r = w0.reshape(N, N, H).transpose(1, 0, 2)          # (j, i, k)
    w0q = np.ascontiguousarray(
        w0r.reshape(8, 4, N, H).reshape(8, 128, H).astype(bf)
    )
    w1r = np.ascontiguousarray(
        w1.reshape(H, N, H).transpose(1, 0, 2).astype(bf)  # (j, i, k)
    )
    w2r = np.ascontiguousarray(
        w2.reshape(H, N, H).transpose(1, 0, 2).astype(bf)
    )
    return x0b, w0q, w1r, w2r


def _get_compiled():
    if "nc" not in _CACHE:
        _CACHE["nc"] = _build_program()
    return _CACHE["nc"]


def run(x0, w0, w1, w2, trace=False):
    nc = _get_compiled()
    x0b, w0q, w1r, w2r = _prep_inputs(
        np.asarray(x0, np.float32),
        np.asarray(w0, np.float32),
        np.asarray(w1, np.float32),
        np.asarray(w2, np.float32),
    )
    core_ids = list(range(NCORES))
    quad_rows = np.repeat(np.arange(N), 32).reshape(8, 128)
    in_maps = []
    for c in core_ids:
        shard = np.ascontiguousarray(x0b[c * BC : (c + 1) * BC])
        x0t = np.ascontiguousarray(shard.transpose(1, 0, 2).reshape(N, COLS))
        in_maps.append(
            {
                "xr": np.ascontiguousarray(np.tile(x0t, (4, 1))),
                "f4a": np.ascontiguousarray(x0t[quad_rows]),
                "fja": np.ascontiguousarray(
                    np.broadcast_to(x0t[:, None, :], (N, 128, COLS))
                ),
                "w0p": w0q,
                "w1p": w1r,
                "w2p": w2r,
            }
        )
    res = run_bass_kernel_spmd(nc, in_maps, core_ids, trace=trace)
    outs = [np.asarray(res.results[c]["out"], np.float32) for c in core_ids]
    return np.concatenate(outs, axis=0), res


def kernel(x0, w0, w1, w2):
    full, _ = run(x0, w0, w1, w2, trace=False)
    return full
